# revision 4
# baseline (speedup 1.0000x reference)
"""Trainium2 Bass kernel for nn_BasicRGCN (2-layer RGCN + DistMult scoring).

Distribution strategy (8 NeuronCores, one chip):
  - Graph-row sharding: core k owns rows [512k, 512k+512) of the node set.
    Each core computes its row-chunk of both RGCN layers over ALL relations,
    accumulating the relation sum exactly in fp32 PSUM (no AllReduce needed).
  - Between layers, the per-core H1 chunks are AllGathered (as fp16 hi/lo
    pairs, 0.25 MiB per rank each) so every core has the full H1 for layer 2.
  - c is folded into A on the host (diag(c_r) A_r H W_r^T == c_r * (A_r H W_r^T)).
  - Matmul precision: A and H are split into fp16 hi+lo pairs on the host
    (A) / device (H1); each A@H product runs as 3 fp16 passes
    (hi*hi + lo*hi + hi*lo) accumulated in fp32 PSUM -> ~fp32 accuracy at
    1 cycle/row (vs 4 cycles/row for native fp32 matmul), with identical
    HBM traffic to fp32 (2+2 bytes/element).
  - The tiny W-projection matmuls run in native fp32 (exact).
  - DistMult scoring (0.01% of the FLOPs, gather-bound) runs on the host
    from the device-computed H2 in float64, then sigmoid.
"""

import numpy as np

R, N, F, B = 8, 4096, 256, 16384
N_CORES = 8
CH = N // N_CORES          # 512 rows per core
KT = N // 128              # 32 contraction k-tiles
G = 8                      # k-tiles per A-stream DMA group
NG = KT // G               # 4 groups
NT = CH // 128             # 4 output row-tiles per chunk

_programs = {}


def _build(reps=1):
    import concourse.bacc as bacc
    import concourse.tile as tile
    import concourse.mybir as mybir

    f16 = mybir.dt.float16
    f32 = mybir.dt.float32

    nc = bacc.Bacc("TRN2", target_bir_lowering=False, debug=False,
                   num_devices=N_CORES)

    a_hi_d = nc.dram_tensor("a_hi", [R, N, CH], f16, kind="ExternalInput")
    a_lo_d = nc.dram_tensor("a_lo", [R, N, CH], f16, kind="ExternalInput")
    h0_hi_d = nc.dram_tensor("h0_hi", [N, F], f16, kind="ExternalInput")
    h0_lo_d = nc.dram_tensor("h0_lo", [N, F], f16, kind="ExternalInput")
    w1t_d = nc.dram_tensor("w1t", [R, F, F], f32, kind="ExternalInput")
    w2t_d = nc.dram_tensor("w2t", [R, F, F], f32, kind="ExternalInput")
    h2_d = nc.dram_tensor("h2", [CH, F], f32, kind="ExternalOutput")

    groups = [list(range(N_CORES))]

    with tile.TileContext(nc) as tc:
        with (
            tc.tile_pool(name="hpool", bufs=1) as hpool,
            tc.tile_pool(name="ahip", bufs=4) as ahip,
            tc.tile_pool(name="alop", bufs=4) as alop,
            tc.tile_pool(name="wpool", bufs=1) as wpool,
            tc.tile_pool(name="ahtp", bufs=2) as ahtp,
            tc.tile_pool(name="hout", bufs=1) as hout,
            tc.tile_pool(name="ps_aht", bufs=4, space="PSUM") as ps_aht,
            tc.tile_pool(name="ps_y", bufs=1, space="PSUM") as ps_y,
            tc.tile_pool(name="dram", bufs=1, space="DRAM") as dram,
        ):
            # persistent: layer-1 H tiles (host-split fp16) and both W tensors
            hh0 = hpool.tile([128, KT, F], f16, tag="h0h")
            hl0 = hpool.tile([128, KT, F], f16, tag="h0l")
            nc.sync.dma_start(hh0[:], h0_hi_d.rearrange("(kt p) f -> p kt f", p=128)[:])
            nc.sync.dma_start(hl0[:], h0_lo_d.rearrange("(kt p) f -> p kt f", p=128)[:])
            w1 = wpool.tile([128, R, 2, F], f32, tag="w1")
            w2 = wpool.tile([128, R, 2, F], f32, tag="w2")
            nc.sync.dma_start(w1[:], w1t_d.rearrange("r (ft p) o -> p r ft o", p=128)[:])
            nc.sync.dma_start(w2[:], w2t_d.rearrange("r (ft p) o -> p r ft o", p=128)[:])

            def emit_layer(h_hi_t, h_lo_t, w_t):
                y_ps = [ps_y.tile([128, F], f32, tag=f"y{nt}", name=f"y{nt}") for nt in range(NT)]

                def emit_y(r, aht_s):
                    for nt in range(NT):
                        ns = slice(nt * 128, nt * 128 + 128)
                        for ft in range(2):
                            nc.tensor.matmul(
                                y_ps[nt][:],
                                aht_s[:, ft, ns],
                                w_t[:, r, ft, :],
                                start=(r == 0 and ft == 0),
                                stop=(r == R - 1 and ft == 1),
                            )

                pending = None
                for r in range(R):
                    ah = []
                    al = []
                    for g in range(NG):
                        th = ahip.tile([128, G, CH], f16, tag="ah")
                        tl = alop.tile([128, G, CH], f16, tag="al")
                        src_h = a_hi_d[r].rearrange("(kt p) n -> p kt n", p=128)
                        src_l = a_lo_d[r].rearrange("(kt p) n -> p kt n", p=128)
                        nc.sync.dma_start(th[:], src_h[:, g * G:(g + 1) * G, :])
                        nc.sync.dma_start(tl[:], src_l[:, g * G:(g + 1) * G, :])
                        ah.append(th)
                        al.append(tl)

                    aht_ps = [ps_aht.tile([128, CH], f32, tag="aht", name=f"aht{r}_{ft2}") for ft2 in range(2)]
                    for ft in range(2):
                        fs = slice(ft * 128, ft * 128 + 128)
                        for kt in range(KT):
                            g, kk = divmod(kt, G)
                            nc.tensor.matmul(aht_ps[ft][:], h_hi_t[:, kt, fs],
                                             ah[g][:, kk, :], start=(kt == 0), stop=False)
                            nc.tensor.matmul(aht_ps[ft][:], h_hi_t[:, kt, fs],
                                             al[g][:, kk, :], start=False, stop=False)
                            nc.tensor.matmul(aht_ps[ft][:], h_lo_t[:, kt, fs],
                                             ah[g][:, kk, :], start=False, stop=(kt == KT - 1))
                    aht_s = ahtp.tile([128, 2, CH], f32, tag="aht_s")
                    for ft in range(2):
                        nc.vector.tensor_copy(aht_s[:, ft, :], aht_ps[ft][:])
                    if pending is not None:
                        emit_y(*pending)
                    pending = (r, aht_s)
                emit_y(*pending)
                return y_ps

            for _rep in range(reps):
                # ---- layer 1 ----
                y_ps = emit_layer(hh0, hl0, w1)
                h1f = hout.tile([128, NT, F], f32, tag="h1f")
                for nt in range(NT):
                    nc.vector.tensor_copy(h1f[:, nt, :], y_ps[nt][:])
                h1h = hout.tile([128, NT, F], f16, tag="h1h")
                nc.vector.tensor_copy(h1h[:], h1f[:])
                h1h32 = hout.tile([128, NT, F], f32, tag="h1h32")
                nc.vector.tensor_copy(h1h32[:], h1h[:])
                h1l = hout.tile([128, NT, F], f16, tag="h1l")
                nc.vector.tensor_sub(h1l[:], h1f[:], h1h32[:])

                bh = dram.tile([CH, F], f16, tag="bh")
                bl = dram.tile([CH, F], f16, tag="bl")
                nc.sync.dma_start(bh.rearrange("(nt p) f -> p nt f", p=128)[:], h1h[:])
                nc.sync.dma_start(bl.rearrange("(nt p) f -> p nt f", p=128)[:], h1l[:])
                gh = dram.tile([N, F], f16, tag="gh", addr_space="Shared")
                gl = dram.tile([N, F], f16, tag="gl", addr_space="Shared")
                nc.gpsimd.collective_compute(
                    "AllGather", mybir.AluOpType.bypass,
                    replica_groups=groups, ins=[bh.opt()], outs=[gh.opt()])
                nc.gpsimd.collective_compute(
                    "AllGather", mybir.AluOpType.bypass,
                    replica_groups=groups, ins=[bl.opt()], outs=[gl.opt()])

                hh1 = hpool.tile([128, KT, F], f16, tag="h1ht")
                hl1 = hpool.tile([128, KT, F], f16, tag="h1lt")
                nc.sync.dma_start(hh1[:], gh.rearrange("(kt p) f -> p kt f", p=128)[:])
                nc.sync.dma_start(hl1[:], gl.rearrange("(kt p) f -> p kt f", p=128)[:])

                # ---- layer 2 ----
                y_ps2 = emit_layer(hh1, hl1, w2)
                h2f = hout.tile([128, NT, F], f32, tag="h2f")
                for nt in range(NT):
                    nc.vector.tensor_copy(h2f[:, nt, :], y_ps2[nt][:])
                nc.sync.dma_start(h2_d.rearrange("(nt p) f -> p nt f", p=128)[:], h2f[:])

    nc.compile()
    return nc


def _get_program(reps=1):
    if reps not in _programs:
        _programs[reps] = _build(reps)
    return _programs[reps]


def _split16(x):
    hi = x.astype(np.float16)
    lo = (x - hi.astype(np.float32)).astype(np.float16)
    return hi, lo


def _prepare_in_maps(adjacency, features, c, W1, W2):
    h0_hi, h0_lo = _split16(np.ascontiguousarray(features, dtype=np.float32))
    w1t = np.ascontiguousarray(W1.transpose(0, 2, 1), dtype=np.float32)
    w2t = np.ascontiguousarray(W2.transpose(0, 2, 1), dtype=np.float32)

    in_maps = []
    for k in range(N_CORES):
        ch = slice(k * CH, (k + 1) * CH)
        a_hi = np.empty((R, N, CH), dtype=np.float16)
        a_lo = np.empty((R, N, CH), dtype=np.float16)
        for r in range(R):
            blk = adjacency[r, ch, :] * c[r, ch, :]          # [CH, N] fp32
            blkT = np.ascontiguousarray(blk.T, dtype=np.float32)  # [N, CH]
            hi, lo = _split16(blkT)
            a_hi[r] = hi
            a_lo[r] = lo
        in_maps.append({
            "a_hi": a_hi, "a_lo": a_lo,
            "h0_hi": h0_hi, "h0_lo": h0_lo,
            "w1t": w1t, "w2t": w2t,
        })
    return in_maps


def _run_device(in_maps, reps=1):
    from concourse.bass_utils import run_bass_kernel_spmd
    nc = _get_program(reps)
    res = run_bass_kernel_spmd(nc, in_maps, core_ids=list(range(N_CORES)))
    return np.concatenate([res.results[k]["h2"] for k in range(N_CORES)], axis=0)


def _score_host(H2, rel_mats, e1_idx, rel_idx, e2_idx):
    E1 = H2[e1_idx].astype(np.float64)
    E2 = H2[e2_idx].astype(np.float64)
    Mm = np.asarray(rel_mats, dtype=np.float64)
    idx = np.arange(F)
    offdiag = Mm.copy()
    offdiag[:, idx, idx] = 0.0
    if not offdiag.any():
        mdiag = Mm[:, idx, idx]
        scores = np.einsum("bf,bf,bf->b", E1, mdiag[rel_idx], E2)
    else:
        scores = np.empty(E1.shape[0], dtype=np.float64)
        for r in range(R):
            m = rel_idx == r
            if m.any():
                scores[m] = np.einsum("bf,fg,bg->b", E1[m], Mm[r], E2[m])
    with np.errstate(over="ignore"):
        out = np.where(scores >= 0,
                       1.0 / (1.0 + np.exp(-scores)),
                       np.exp(scores) / (1.0 + np.exp(scores)))
    return out.astype(np.float32)


def kernel(adjacency, features, c, W1, W2, rel_mats, e1_idx, rel_idx, e2_idx,
           _reps=1):
    adjacency = np.asarray(adjacency, dtype=np.float32)
    features = np.asarray(features, dtype=np.float32)
    c = np.asarray(c, dtype=np.float32)
    W1 = np.asarray(W1, dtype=np.float32)
    W2 = np.asarray(W2, dtype=np.float32)
    rel_mats = np.asarray(rel_mats, dtype=np.float32)
    e1_idx = np.asarray(e1_idx)
    rel_idx = np.asarray(rel_idx)
    e2_idx = np.asarray(e2_idx)

    in_maps = _prepare_in_maps(adjacency, features, c, W1, W2)
    H2 = _run_device(in_maps, reps=_reps)
    return _score_host(H2, rel_mats, e1_idx, rel_idx, e2_idx)
